# revision 5
# baseline (speedup 1.0000x reference)
"""Trainium2 Bass kernel: dense multi-head attention layer (B=4, L=S=2048,
d_model=1024, 16 heads x 64).

Sharding: 8 cores = (batch b in 0..3) x (query-half in 0..1). Each core runs
the full 16-head attention for its 1024 query rows (K/V projection duplicated
between the two cores sharing a batch) plus the out-projection for those rows,
so no collectives are needed and each core emits complete output rows.

Layout trick: inputs are pre-transposed on the host to [d_model, pos] so every
matmul in the chain is natural for the PE array (contraction on partitions):
  qT = Wq.T @ XqT          (lhsT=Wq,  rhs=XqT)        -> [fout, pos]
  kT = Wk.T @ XkT          (lhsT=Wk,  rhs=XkT)        -> [fout, pos]
  v  = XvT.T @ Wv          (lhsT=XvT, rhs=Wv)         -> [pos, fout]
  scoresT_h = kT_h.T @ qT_h  (K=64)                   -> [S, L]
  exp on ACT (no max subtraction needed: logits ~ N(0,1))
  avT_h = [v_h | 1].T @ expT_h  (K=S tiles, M=65)     -> [E+1, L], row 64 = sums
  outT_h = avT_h[0:64] * (1/sums broadcast)           -> [E, L]
  final = outT.T @ Wo (lhsT=outT, rhs=Wo)             -> [L, fout]  natural!
Biases enter via per-partition ACT/DVE adds (bq, bk) or K=1 ones-row matmuls
(bv, bo). Softmax scale 1/8 is folded into the ACT exp scale.
"""

import os

import ml_dtypes
import numpy as np

B, L, S, DM, H, E = 4, 2048, 2048, 1024, 16, 64
P = 128
FT = DM // P          # 8 feature tiles of 128
LL = L // 2           # 1024 query rows per core
NST = S // P          # 16 key/value position tiles
NH2 = H // 2          # 8 head pairs
W = E + 1             # v columns + ones column for the softmax denominator
NCORES = 8

_graph_cache = {}


def _build_graph():
    if "nc" in _graph_cache:
        return _graph_cache["nc"]

    import concourse.tile as tile
    from concourse import bacc, mybir

    BF16 = mybir.dt.bfloat16
    F32 = mybir.dt.float32
    Exp = mybir.ActivationFunctionType.Exp

    nc = bacc.Bacc("TRN2", target_bir_lowering=False, debug=False,
                   num_devices=NCORES)

    xqT_d = nc.declare_dram_parameter("xqT", [DM, LL], BF16, isOutput=False)
    xkT_d = nc.declare_dram_parameter("xkT", [DM, S], BF16, isOutput=False)
    xvT_d = nc.declare_dram_parameter("xvT", [DM, S], BF16, isOutput=False)
    wq_d = nc.declare_dram_parameter("wq", [DM, DM], BF16, isOutput=False)
    wk_d = nc.declare_dram_parameter("wk", [DM, DM], BF16, isOutput=False)
    wv_d = nc.declare_dram_parameter("wv", [DM, DM], BF16, isOutput=False)
    wo_d = nc.declare_dram_parameter("wo", [DM, DM], BF16, isOutput=False)
    bq_d = nc.declare_dram_parameter("bq", [DM], F32, isOutput=False)
    bk_d = nc.declare_dram_parameter("bk", [DM], F32, isOutput=False)
    bv_d = nc.declare_dram_parameter("bv", [DM], BF16, isOutput=False)
    bo_d = nc.declare_dram_parameter("bo", [DM], BF16, isOutput=False)
    out_d = nc.declare_dram_parameter("out", [LL, DM], F32, isOutput=True)

    NC_Q = LL // 512      # 2 query-position chunks of 512
    NC_S = S // 512       # 4 key-position chunks of 512

    with tile.TileContext(nc) as tc:
        with tc.tile_pool(name="persist", bufs=1) as pp, \
             tc.tile_pool(name="norm", bufs=2) as npool:
            qT = pp.tile([P, FT, LL], BF16)
            kT = pp.tile([P, FT, S], BF16)
            v_sb = pp.tile([P, NST, H * W], BF16)
            outT = pp.tile([P, FT, LL], BF16)
            bq_sb = pp.tile([P, FT], F32)
            bk_sb = pp.tile([P, FT], F32)
            bv_sb = pp.tile([1, DM], BF16)
            bo_sb = pp.tile([1, DM], BF16)
            ones_bf = pp.tile([1, P], BF16)

            nc.vector.memset(ones_bf, 1.0)
            v4 = v_sb[:].rearrange("p s (h w) -> p s h w", w=W)
            nc.vector.memset(v4[:, :, :, E:W], 1.0)

            nc.sync.dma_start(out=bq_sb[:],
                              in_=bq_d.ap().rearrange("(f p) -> p f", p=P))
            nc.sync.dma_start(out=bk_sb[:],
                              in_=bk_d.ap().rearrange("(f p) -> p f", p=P))
            nc.sync.dma_start(out=bv_sb[:],
                              in_=bv_d.ap().rearrange("(a d) -> a d", a=1))
            nc.sync.dma_start(out=bo_sb[:],
                              in_=bo_d.ap().rearrange("(a d) -> a d", a=1))

            # ---------------- Phase A: QKV projections ----------------
            with tc.tile_pool(name="psA", bufs=6, space="PSUM") as psA:
                # Q: qT[fout, pos] += bq
                with tc.tile_pool(name="a1", bufs=1) as a1:
                    wq_sb = a1.tile([P, FT, DM], BF16)
                    xq_sb = a1.tile([P, FT, LL], BF16)
                    nc.sync.dma_start(
                        out=wq_sb[:],
                        in_=wq_d.ap().rearrange("(f p) n -> p f n", p=P))
                    nc.sync.dma_start(
                        out=xq_sb[:],
                        in_=xqT_d.ap().rearrange("(f p) n -> p f n", p=P))
                    for m in range(FT):
                        pss = [psA.tile([P, 512], F32, tag="psa", name=f"psa_q{m}_{c}")
                               for c in range(NC_Q)]
                        for f in range(FT):
                            for c in range(NC_Q):
                                nc.tensor.matmul(
                                    pss[c][:],
                                    lhsT=wq_sb[:, f, m * P:(m + 1) * P],
                                    rhs=xq_sb[:, f, c * 512:(c + 1) * 512],
                                    start=(f == 0), stop=(f == FT - 1))
                        for c in range(NC_Q):
                            nc.vector.tensor_scalar_add(
                                qT[:, m, c * 512:(c + 1) * 512],
                                pss[c][:], bq_sb[:, m:m + 1])

                # K: kT[fout, pos] += bk
                with tc.tile_pool(name="a2", bufs=1) as a2:
                    wk_sb = a2.tile([P, FT, DM], BF16)
                    xk_sb = a2.tile([P, FT, S], BF16)
                    nc.sync.dma_start(
                        out=wk_sb[:],
                        in_=wk_d.ap().rearrange("(f p) n -> p f n", p=P))
                    nc.sync.dma_start(
                        out=xk_sb[:],
                        in_=xkT_d.ap().rearrange("(f p) n -> p f n", p=P))
                    for m in range(FT):
                        pss = [psA.tile([P, 512], F32, tag="psa", name=f"psa_k{m}_{c}")
                               for c in range(NC_S)]
                        for f in range(FT):
                            for c in range(NC_S):
                                nc.tensor.matmul(
                                    pss[c][:],
                                    lhsT=wk_sb[:, f, m * P:(m + 1) * P],
                                    rhs=xk_sb[:, f, c * 512:(c + 1) * 512],
                                    start=(f == 0), stop=(f == FT - 1))
                        for c in range(NC_S):
                            nc.vector.tensor_scalar_add(
                                kT[:, m, c * 512:(c + 1) * 512],
                                pss[c][:], bk_sb[:, m:m + 1])

                # V: v[pos, fout] += bv (ones-row matmul), stored with a ones
                # column appended per head block for the softmax denominator.
                with tc.tile_pool(name="a3", bufs=1) as a3:
                    wv_sb = a3.tile([P, FT, DM], BF16)
                    xv_sb = a3.tile([P, FT, S], BF16)
                    nc.sync.dma_start(
                        out=wv_sb[:],
                        in_=wv_d.ap().rearrange("(f p) n -> p f n", p=P))
                    nc.sync.dma_start(
                        out=xv_sb[:],
                        in_=xvT_d.ap().rearrange("(f p) n -> p f n", p=P))
                    for t in range(NST):
                        pss = [psA.tile([P, 512], F32, tag="psa", name=f"psa_v{t}_{c}")
                               for c in range(2)]
                        for f in range(FT):
                            for c in range(2):
                                nc.tensor.matmul(
                                    pss[c][:],
                                    lhsT=xv_sb[:, f, t * P:(t + 1) * P],
                                    rhs=wv_sb[:, f, c * 512:(c + 1) * 512],
                                    start=(f == 0), stop=False)
                        for c in range(2):
                            nc.tensor.matmul(
                                pss[c][:],
                                lhsT=ones_bf[:, 0:P],
                                rhs=bv_sb[:, c * 512:(c + 1) * 512],
                                start=False, stop=True)
                            nc.vector.tensor_copy(
                                out=v4[:, t, c * 8:(c + 1) * 8, 0:E],
                                in_=pss[c][:].rearrange("p (h e) -> p h e",
                                                        e=E))

            # ---------------- Phase B: attention ----------------
            with tc.tile_pool(name="wop", bufs=1) as wop:
                wo_sb = wop.tile([P, FT, DM], BF16)
                nc.sync.dma_start(
                    out=wo_sb[:],
                    in_=wo_d.ap().rearrange("(f p) n -> p f n", p=P))

                with tc.tile_pool(name="expp", bufs=3) as expp, \
                     tc.tile_pool(name="ps_sc", bufs=2, space="PSUM") as ps_sc, \
                     tc.tile_pool(name="ps_av", bufs=4, space="PSUM") as ps_av:
                    for j in range(NH2):           # head pair: heads 2j, 2j+1
                        for c in range(NC_Q):      # query chunk of 512
                            avs = [ps_av.tile([W, 512], F32, tag="av", name=f"av_{j}_{c}_{r}")
                                   for r in range(2)]
                            for half in range(2):  # 8 S-tiles per half
                                ex = expp.tile([P, 2, 8, 512], BF16, tag="ex")
                                # scoresT + exp, two heads row-packed
                                for sb in range(4):   # blocks of 2 S-tiles
                                    scp = [ps_sc.tile([P, 2, 512], F32, tag="sc",
                                                      name=f"sc_{j}_{c}_{half}_{sb}_{r}")
                                           for r in range(2)]
                                    for si in range(2):
                                        s = half * 8 + sb * 2 + si
                                        for r in range(2):
                                            nc.tensor.matmul(
                                                scp[r][:, si, :],
                                                lhsT=kT[r * E:(r + 1) * E, j,
                                                        s * P:(s + 1) * P],
                                                rhs=qT[r * E:(r + 1) * E, j,
                                                       c * 512:(c + 1) * 512],
                                                start=True, stop=True)
                                    for r in range(2):
                                        nc.scalar.activation(
                                            out=ex[:, r, sb * 2:sb * 2 + 2, :],
                                            in_=scp[r][:],
                                            func=Exp, scale=0.125)
                                # A @ V for this half (accumulate over S)
                                for st in range(8):
                                    s = half * 8 + st
                                    for r in range(2):
                                        nc.tensor.matmul(
                                            avs[r][:],
                                            lhsT=v4[:, s, 2 * j + r, :],
                                            rhs=ex[:, r, st, :],
                                            start=(s == 0), stop=(s == NST - 1))
                            # normalize: row E of avs holds the exp row-sums
                            for r in range(2):
                                recip = npool.tile([1, 512], F32, tag="recip")
                                nc.vector.reciprocal(recip[:],
                                                     avs[r][E:W, :])
                                bc = npool.tile([E, 512], F32, tag="bc")
                                nc.gpsimd.partition_broadcast(bc[:], recip[:])
                                nc.vector.tensor_mul(
                                    outT[r * E:(r + 1) * E, j,
                                         c * 512:(c + 1) * 512],
                                    avs[r][0:E, :], bc[:])

                # ---------------- Phase C: out projection ----------------
                with tc.tile_pool(name="psC", bufs=4, space="PSUM") as psC, \
                     tc.tile_pool(name="osb", bufs=4) as osb:
                    for t in range(LL // P):      # 8 query-row tiles
                        for c2 in range(2):       # fout chunks of 512
                            ps = psC.tile([P, 512], F32, tag="psc")
                            for f in range(FT):
                                nc.tensor.matmul(
                                    ps[:],
                                    lhsT=outT[:, f, t * P:(t + 1) * P],
                                    rhs=wo_sb[:, f, c2 * 512:(c2 + 1) * 512],
                                    start=(f == 0), stop=False)
                            nc.tensor.matmul(
                                ps[:], lhsT=ones_bf[:, 0:P],
                                rhs=bo_sb[:, c2 * 512:(c2 + 1) * 512],
                                start=False, stop=True)
                            o_sb = osb.tile([P, 512], F32, tag="osb")
                            nc.scalar.copy(o_sb[:], ps[:])
                            nc.sync.dma_start(
                                out=out_d[t * P:(t + 1) * P,
                                          c2 * 512:(c2 + 1) * 512],
                                in_=o_sb[:])

    nc.finalize()
    _graph_cache["nc"] = nc
    return nc


def _install_profile_shim():
    """Provide antenv.axon_hooks (NTFF capture via libaxon_pjrt ctypes) when
    the image's antenv lacks it, and skip the artifact upload step."""
    import contextlib
    import ctypes
    import sys
    import types

    try:
        from antenv.axon_hooks import get_axon_ntff_profile_hook
        if get_axon_ntff_profile_hook() is not None:
            return
    except ImportError:
        pass

    so_path = "/opt/axon/libaxon_pjrt.so"
    try:
        lib = ctypes.CDLL(so_path)
    except OSError:
        return
    if not hasattr(lib, "axon_start_nrt_profile"):
        return
    lib.axon_start_nrt_profile.argtypes = [ctypes.POINTER(ctypes.c_int64),
                                           ctypes.c_size_t]
    lib.axon_start_nrt_profile.restype = ctypes.c_int64
    lib.axon_stop_nrt_profile.argtypes = [ctypes.c_char_p]
    lib.axon_stop_nrt_profile.restype = ctypes.c_int64

    @contextlib.contextmanager
    def _hook(output_dir, device_ids):
        import jax
        jax.devices()
        if device_ids:
            ids = (ctypes.c_int64 * len(device_ids))(*device_ids)
            rc = lib.axon_start_nrt_profile(ids, len(device_ids))
        else:
            rc = lib.axon_start_nrt_profile(None, 0)
        if rc != 0:
            raise RuntimeError(f"axon_start_nrt_profile rc={rc}")
        try:
            yield
        finally:
            n = lib.axon_stop_nrt_profile(str(output_dir).encode())
            print(f"profile: {n} file(s) written to {output_dir}",
                  file=sys.stderr)

    mod = types.ModuleType("antenv.axon_hooks")
    mod.get_axon_ntff_profile_hook = lambda: _hook
    mod.set_axon_ntff_profile_hook = lambda h: None
    sys.modules["antenv.axon_hooks"] = mod

    import concourse.bass_utils as bu
    bu.upload_artifacts = lambda tmpdir: str(tmpdir)


def kernel(queries, keys, values, Wq, bq, Wk, bk, Wv, bv, Wo, bo):
    from concourse.bass_utils import run_bass_kernel_spmd

    nc = _build_graph()
    bf = ml_dtypes.bfloat16

    wq_b = np.ascontiguousarray(np.asarray(Wq, np.float32).astype(bf))
    wk_b = np.ascontiguousarray(np.asarray(Wk, np.float32).astype(bf))
    wv_b = np.ascontiguousarray(np.asarray(Wv, np.float32).astype(bf))
    wo_b = np.ascontiguousarray(np.asarray(Wo, np.float32).astype(bf))
    bq_f = np.ascontiguousarray(np.asarray(bq, np.float32))
    bk_f = np.ascontiguousarray(np.asarray(bk, np.float32))
    bv_b = np.ascontiguousarray(np.asarray(bv, np.float32).astype(bf))
    bo_b = np.ascontiguousarray(np.asarray(bo, np.float32).astype(bf))

    qT = np.ascontiguousarray(
        np.transpose(np.asarray(queries, np.float32), (0, 2, 1)).astype(bf))
    kTt = np.ascontiguousarray(
        np.transpose(np.asarray(keys, np.float32), (0, 2, 1)).astype(bf))
    vTt = np.ascontiguousarray(
        np.transpose(np.asarray(values, np.float32), (0, 2, 1)).astype(bf))

    in_maps = []
    for core in range(NCORES):
        b, h = divmod(core, 2)
        in_maps.append({
            "xqT": np.ascontiguousarray(qT[b][:, h * LL:(h + 1) * LL]),
            "xkT": kTt[b],
            "xvT": vTt[b],
            "wq": wq_b, "wk": wk_b, "wv": wv_b, "wo": wo_b,
            "bq": bq_f, "bk": bk_f, "bv": bv_b, "bo": bo_b,
        })

    trace = bool(int(os.environ.get("KERNEL_PROFILE", "0")))
    if trace:
        _install_profile_shim()
    res = run_bass_kernel_spmd(nc, in_maps, core_ids=list(range(NCORES)),
                               trace=trace)
    kernel.last_results = res

    out = np.empty((B, L, DM), np.float32)
    for core in range(NCORES):
        b, h = divmod(core, 2)
        out[b, h * LL:(h + 1) * LL, :] = res.results[core]["out"]
    return out


kernel.last_results = None
